# revision 12
# baseline (speedup 1.0000x reference)
"""Trainium2 Bass kernel for nn_CruxMiniCircuit (gnn_message_passing).

Reference semantics: B independent rows; each row is a circuit of N nodes
(literal nodes hold a fixed one-hot distribution over 10 ints, op nodes
combine left/right child distributions through a per-op bilinear table
followed by softmax).  The reference runs 10 synchronous passes over all
nodes and returns only the root (node 0) logits per row.

Key observation: the output depends only on node 0's dependency cone
unrolled 10 passes deep.  Literal children are compile-time constants
(one-hot vectors) and op nodes at pass 0 are zero, so the per-row
worklists are tiny (mean ~5 updates/row, max ~80 for the benchmark
distribution).  The host precomputes integer worklists / gather indices;
the device performs all floating-point work: for each pass, gather child
value vectors, form outer products and the bilinear contraction on the
TensorEngine, softmax via exp + PE ones-matmul reduction, and store all
three per-op results so op selection folds into the next pass's gather
indexing.

Sharding: pure data parallel over the batch dim (B=2048 -> 256 rows on
each of the 8 NeuronCores), as per the sharding hint.  No collectives are
needed for the forward pass.
"""

import sys
from contextlib import ExitStack

import numpy as np

sys.path.insert(0, "/opt/trn_rl_repo")

import concourse.bass as bass
import concourse.tile as tile
from concourse import bacc, mybir
from concourse.bass_utils import run_bass_kernel_spmd

B, N = 2048, 1023
NI, NO, NP = 10, 3, 10  # n_ints, n_ops, n_passes
NCORES = 8
RPC = B // NCORES  # rows per core
ZSLOT = NI  # value-buffer slot holding the zero vector
NCONST = NI + 1  # slots 0..9 = one-hot e_k, slot 10 = zeros
CHUNK = 448  # free-dim chunk for the compute pipeline (PSUM/matmul limits)

TRACE = False  # set True (e.g. from test.py) to profile the HW run
DEBUG_BUF = False  # export the SBUF value buffer for debugging
LAST_RESULTS = None  # BassKernelResults of the last run (exec_time_ns etc.)


def _plan(cats, ops, lits, left, right, mask):
    """Integer-only preprocessing: worklists, slots, gather indices."""
    left = np.clip(left.astype(np.int64), 0, N - 1)
    right = np.clip(right.astype(np.int64), 0, N - 1)
    opsc = np.clip(ops.astype(np.int64), 0, NO - 1)
    litsc = np.clip(lits.astype(np.int64), 0, NI - 1)
    m = mask.astype(bool)
    is_lit = (cats == 0) & m
    is_opa = (cats == 1) & m  # active op nodes (get updated each pass)
    # Value of any non-(active-op) node, as a constant-region slot index.
    const_slot = np.where(is_lit, litsc, ZSLOT)

    # Worklists W[p]: the (row, node) updates that must be computed at pass p.
    # W[NP] = root per row with cats==1 (node_logits needed regardless of mask);
    # W[p-1] = active-op children of W[p].
    Wr = [None] * (NP + 1)
    Wn = [None] * (NP + 1)
    r10 = np.nonzero(cats[:, 0] == 1)[0].astype(np.int64)
    Wr[NP], Wn[NP] = r10, np.zeros(len(r10), np.int64)
    need = np.zeros((B, N), bool)
    for p in range(NP, 1, -1):
        r, n = Wr[p], Wn[p]
        cr = np.concatenate([r, r])
        cn = np.concatenate([left[r, n], right[r, n]])
        keep = is_opa[cr, cn]
        need[:] = False
        need[cr[keep], cn[keep]] = True
        rr, nn = np.nonzero(need)
        Wr[p - 1], Wn[p - 1] = rr.astype(np.int64), nn.astype(np.int64)

    # Per-core padded sizes (uniform across cores; one NEFF for all).
    Pp = np.zeros(NP + 1, np.int64)
    seg = [None] * (NP + 1)  # per pass: array of core segment starts (len NCORES+1)
    for p in range(1, NP + 1):
        s = np.searchsorted(Wr[p], np.arange(NCORES + 1) * RPC)
        seg[p] = s
        mx = int((s[1:] - s[:-1]).max()) if len(Wr[p]) else 0
        Pp[p] = max(8, -(-mx // 8) * 8)  # multiple of 8 so num_idxs % 16 == 0

    # Buffer slot bases: passes 1..NP-1 store 3 variants per update.
    base = np.zeros(NP + 1, np.int64)
    base[1] = NCONST
    for p in range(2, NP + 1):
        base[p] = base[p - 1] + 3 * Pp[p - 1]
    S = int(base[NP - 1] + 3 * Pp[NP - 1]) if NP >= 2 else NCONST
    assert S <= 32000, f"value buffer too large for int16 gather indices: {S}"

    # Per-(row,node) local slot index within its core's pass-p list.
    def local_ids(p):
        r = Wr[p]
        out = np.arange(len(r), dtype=np.int64)
        out -= seg[p][r // RPC]
        return out

    # Gather index arrays per pass, wrapped for ap_gather.
    idx_wrapped = []
    Ftot = 0
    slot_prev = np.full((B, N), -1, np.int64)
    for p in range(1, NP + 1):
        r, n = Wr[p], Wn[p]
        lj = local_ids(p)
        lch, rch = left[r, n], right[r, n]
        if p == 1:
            lidx = const_slot[r, lch]
            ridx = const_slot[r, rch]
        else:
            lidx = np.where(
                is_opa[r, lch],
                base[p - 1] + 3 * slot_prev[r, lch] + opsc[r, lch],
                const_slot[r, lch],
            )
            ridx = np.where(
                is_opa[r, rch],
                base[p - 1] + 3 * slot_prev[r, rch] + opsc[r, rch],
                const_slot[r, rch],
            )
        P = int(Pp[p])
        arr = np.full((NCORES, 2 * P), ZSLOT, np.int64)
        core = r // RPC
        arr[core, lj] = lidx
        arr[core, P + lj] = ridx
        F = -(-2 * P // 16)
        F += F & 1  # keep each pass idx slice 4-byte aligned (ucode reads dwords)
        tmp = np.full((NCORES, F * 16), ZSLOT, np.int64)
        tmp[:, : 2 * P] = arr
        idx_wrapped.append(tmp.reshape(NCORES, F, 16).transpose(0, 2, 1).astype(np.int16))
        Ftot += F
        if p < NP:
            slot_prev = np.full((B, N), -1, np.int64)
            slot_prev[r, n] = lj

    idx_full = np.concatenate(idx_wrapped, axis=2)  # (NCORES, 16, Ftot)

    return dict(
        Pp=Pp, base=base, S=S, idx=idx_full, Ftot=Ftot,
        r10=r10, seg10=seg[NP], lj10=local_ids(NP),
        opsc=opsc, litsc=litsc, is_lit=is_lit, m=m,
    )


def _build_nc(S, Pp, Ftot, P10):
    f32 = mybir.dt.float32
    nc = bacc.Bacc(None)
    consts = nc.dram_tensor("consts", [NI, NCONST], f32, kind="ExternalInput")
    wmat = nc.dram_tensor("wmat", [100, 74], f32, kind="ExternalInput")
    repl = nc.dram_tensor("repl", [NI, 100], f32, kind="ExternalInput")
    reprm = nc.dram_tensor("reprm", [NI, 100], f32, kind="ExternalInput")
    oblk = nc.dram_tensor("oblk", [74, NO], f32, kind="ExternalInput")
    oblk2 = nc.dram_tensor("oblk2", [NO, 74], f32, kind="ExternalInput")
    idx_in = nc.dram_tensor("idx", [16, Ftot], mybir.dt.int16, kind="ExternalInput")
    outz = nc.dram_tensor("outz", [74, P10], f32, kind="ExternalOutput")
    bufout = nc.dram_tensor("bufout", [16, S], f32, kind="ExternalOutput") if DEBUG_BUF else None

    with ExitStack() as ctx:
        tc = ctx.enter_context(tile.TileContext(nc))
        singles = ctx.enter_context(tc.tile_pool(name="singles", bufs=1))
        work = ctx.enter_context(tc.tile_pool(name="work", bufs=2))
        psum = ctx.enter_context(tc.tile_pool(name="psum", bufs=1, space="PSUM"))

        buf = singles.tile([16, S], f32)
        nc.vector.memset(buf[:, :], 0.0)
        nc.sync.dma_start(out=buf[0:NI, 0:NCONST], in_=consts[:, :])
        w_sb = singles.tile([100, 74], f32)
        nc.sync.dma_start(out=w_sb[:, :], in_=wmat[:, :])
        repl_sb = singles.tile([NI, 100], f32)
        nc.sync.dma_start(out=repl_sb[:, :], in_=repl[:, :])
        reprm_sb = singles.tile([NI, 100], f32)
        nc.sync.dma_start(out=reprm_sb[:, :], in_=reprm[:, :])
        oblk_sb = singles.tile([74, NO], f32)
        nc.sync.dma_start(out=oblk_sb[:, :], in_=oblk[:, :])
        oblk2_sb = singles.tile([NO, 74], f32)
        nc.sync.dma_start(out=oblk2_sb[:, :], in_=oblk2[:, :])
        idx_sb = singles.tile([16, Ftot], mybir.dt.int16)
        nc.sync.dma_start(out=idx_sb[:, :], in_=idx_in[:, :])

        foff = 0
        for p in range(1, NP + 1):
            P = int(Pp[p])
            F = -(-2 * P // 16)
            F += F & 1  # match _plan's 4-byte-aligned idx stride
            lr = work.tile([16, 2 * P], f32, tag="lr")
            nc.gpsimd.ap_gather(
                out_ap=lr[:, :],
                in_ap=buf[:, :],
                idxs_ap=idx_sb[:, foff : foff + F],
                channels=16,
                num_elems=S,
                d=1,
                num_idxs=2 * P,
            )
            foff += F
            for c0 in range(0, P, CHUNK):
                cw = min(CHUNK, P - c0)
                ps_l = psum.tile([100, cw], f32, tag="ps_l")
                nc.tensor.matmul(ps_l[:, :], repl_sb[:, :], lr[0:NI, c0 : c0 + cw],
                                 start=True, stop=True)
                ps_r = psum.tile([100, cw], f32, tag="ps_r")
                nc.tensor.matmul(ps_r[:, :], reprm_sb[:, :], lr[0:NI, P + c0 : P + c0 + cw],
                                 start=True, stop=True)
                lrep_sb = work.tile([100, cw], f32, tag="lrep_sb")
                nc.scalar.copy(lrep_sb[:, :], ps_l[:, :])
                outer = work.tile([100, cw], f32, tag="outer")
                nc.vector.tensor_mul(outer[:, :], lrep_sb[:, :], ps_r[:, :])
                ps_z = psum.tile([74, cw], f32, tag="ps_z")
                nc.tensor.matmul(ps_z[:, :], w_sb[:, :], outer[:, :], start=True, stop=True)
                if p == NP:
                    zsb = work.tile([74, cw], f32, tag="zsb")
                    nc.scalar.copy(zsb[:, :], ps_z[:, :])
                    nc.sync.dma_start(out=outz[:, c0 : c0 + cw], in_=zsb[:, :])
                    continue
                e = work.tile([74, cw], f32, tag="e")
                nc.scalar.activation(e[:, :], ps_z[:, :], mybir.ActivationFunctionType.Exp)
                ps_z3 = psum.tile([NO, cw], f32, tag="ps_z3")
                nc.tensor.matmul(ps_z3[:, :], oblk_sb[:, :], e[:, :], start=True, stop=True)
                rz = work.tile([NO, cw], f32, tag="rz")
                nc.vector.reciprocal(rz[:, :], ps_z3[:, :])
                ps_rz = psum.tile([74, cw], f32, tag="ps_rz")
                nc.tensor.matmul(ps_rz[:, :], oblk2_sb[:, :], rz[:, :], start=True, stop=True)
                b0 = int(_CUR_BASE[p]) + 3 * c0
                for o in range(NO):
                    dst = buf[0:NI, b0 + o : b0 + 3 * cw : 3]
                    nc.vector.tensor_mul(
                        dst,
                        e[o * 32 : o * 32 + NI, :],
                        ps_rz[o * 32 : o * 32 + NI, :],
                    )
        if bufout is not None:
            nc.sync.dma_start(out=bufout[:, :], in_=buf[:, :])
    nc.finalize()
    return nc


_CUR_BASE = None  # set by kernel() before _build_nc (slot base per pass)


def kernel(op_table, cats, ops, lits, left, right, mask):
    global _CUR_BASE, LAST_RESULTS
    op_table = np.asarray(op_table, np.float32)
    plan = _plan(np.asarray(cats), np.asarray(ops), np.asarray(lits),
                 np.asarray(left), np.asarray(right), np.asarray(mask))
    Pp, base, S, Ftot = plan["Pp"], plan["base"], plan["S"], plan["Ftot"]
    P10 = int(Pp[NP])
    _CUR_BASE = base

    nc = _build_nc(S, Pp, Ftot, P10)

    consts = np.concatenate([np.eye(NI, dtype=np.float32),
                             np.zeros((NI, 1), np.float32)], axis=1)
    wmat = np.zeros((100, 74), np.float32)
    w30 = op_table.transpose(1, 2, 0, 3).reshape(100, 30)
    oblk = np.zeros((74, NO), np.float32)
    oblk2 = np.zeros((NO, 74), np.float32)
    for o in range(NO):
        wmat[:, o * 32 : o * 32 + NI] = w30[:, o * NI : (o + 1) * NI]
        oblk[o * 32 : o * 32 + NI, o] = 1.0
        oblk2[o, o * 32 : o * 32 + NI] = 1.0
    repl = np.kron(np.eye(NI), np.ones((1, NI))).astype(np.float32)
    reprm = np.tile(np.eye(NI), (1, NI)).astype(np.float32)

    in_maps = []
    for c in range(NCORES):
        in_maps.append({
            "consts": consts, "wmat": wmat, "repl": repl, "reprm": reprm,
            "oblk": oblk, "oblk2": oblk2,
            "idx": np.ascontiguousarray(plan["idx"][c]),
        })

    res = run_bass_kernel_spmd(nc, in_maps, list(range(NCORES)), trace=TRACE)
    LAST_RESULTS = res

    # Assemble the full (B, NI) output on the host (index selection only).
    out = np.zeros((B, NI), np.float32)
    litsc, is_lit, m = plan["litsc"], plan["is_lit"], plan["m"]
    lit_rows = np.nonzero(cats[:, 0] == 0)[0]
    lr_active = is_lit[lit_rows, 0]
    oh = 10.0 * np.eye(NI, dtype=np.float32)[litsc[lit_rows, 0]]
    out[lit_rows] = np.where(lr_active[:, None], oh, 0.0)

    r10, lj10, opsc = plan["r10"], plan["lj10"], plan["opsc"]
    for c in range(NCORES):
        z = np.asarray(res.results[c]["outz"])  # (30, P10)
        s0, s1 = plan["seg10"][c], plan["seg10"][c + 1]
        rows = r10[s0:s1]
        cols = lj10[s0:s1]
        o = opsc[rows, 0]
        zc = z[:, cols]  # (74, cnt)
        sel = np.stack([zc[i * 32 : i * 32 + NI, :] for i in range(NO)])  # (3,10,cnt)
        out[rows] = sel[o, :, np.arange(len(rows))]
    return out


# revision 13
# speedup vs baseline: 1.0166x; 1.0166x over previous
"""Trainium2 Bass kernel for nn_CruxMiniCircuit (gnn_message_passing).

Reference semantics: B independent rows; each row is a circuit of N nodes
(literal nodes hold a fixed one-hot distribution over 10 ints, op nodes
combine left/right child distributions through a per-op bilinear table
followed by softmax).  The reference runs 10 synchronous passes over all
nodes and returns only the root (node 0) logits per row.

Key observation: the output depends only on node 0's dependency cone
unrolled 10 passes deep.  Literal children are compile-time constants
(one-hot vectors) and op nodes at pass 0 are zero, so the per-row
worklists are tiny (mean ~5 updates/row, max ~80 for the benchmark
distribution).  The host precomputes integer worklists / gather indices;
the device performs all floating-point work: for each pass, gather child
value vectors, form outer products and the bilinear contraction on the
TensorEngine, softmax via exp + PE ones-matmul reduction, and store all
three per-op results so op selection folds into the next pass's gather
indexing.

Sharding: pure data parallel over the batch dim (B=2048 -> 256 rows on
each of the 8 NeuronCores), as per the sharding hint.  No collectives are
needed for the forward pass.
"""

import sys
from contextlib import ExitStack

import numpy as np

sys.path.insert(0, "/opt/trn_rl_repo")

import concourse.bass as bass
import concourse.tile as tile
from concourse import bacc, mybir
from concourse.bass_utils import run_bass_kernel_spmd

B, N = 2048, 1023
NI, NO, NP = 10, 3, 10  # n_ints, n_ops, n_passes
NCORES = 8
RPC = B // NCORES  # rows per core
ZSLOT = NI  # value-buffer slot holding the zero vector
NCONST = NI + 1  # slots 0..9 = one-hot e_k, slot 10 = zeros
CHUNK = 448  # free-dim chunk for the compute pipeline (PSUM/matmul limits)

TRACE = False  # set True (e.g. from test.py) to profile the HW run
DEBUG_BUF = False  # export the SBUF value buffer for debugging
LAST_RESULTS = None  # BassKernelResults of the last run (exec_time_ns etc.)


def _plan(cats, ops, lits, left, right, mask):
    """Integer-only preprocessing: worklists, slots, gather indices."""
    left = np.clip(left.astype(np.int64), 0, N - 1)
    right = np.clip(right.astype(np.int64), 0, N - 1)
    opsc = np.clip(ops.astype(np.int64), 0, NO - 1)
    litsc = np.clip(lits.astype(np.int64), 0, NI - 1)
    m = mask.astype(bool)
    is_lit = (cats == 0) & m
    is_opa = (cats == 1) & m  # active op nodes (get updated each pass)
    # Value of any non-(active-op) node, as a constant-region slot index.
    const_slot = np.where(is_lit, litsc, ZSLOT)

    # Worklists W[p]: the (row, node) updates that must be computed at pass p.
    # W[NP] = root per row with cats==1 (node_logits needed regardless of mask);
    # W[p-1] = active-op children of W[p].
    Wr = [None] * (NP + 1)
    Wn = [None] * (NP + 1)
    r10 = np.nonzero(cats[:, 0] == 1)[0].astype(np.int64)
    Wr[NP], Wn[NP] = r10, np.zeros(len(r10), np.int64)
    need = np.zeros((B, N), bool)
    for p in range(NP, 1, -1):
        r, n = Wr[p], Wn[p]
        cr = np.concatenate([r, r])
        cn = np.concatenate([left[r, n], right[r, n]])
        keep = is_opa[cr, cn]
        need[:] = False
        need[cr[keep], cn[keep]] = True
        rr, nn = np.nonzero(need)
        Wr[p - 1], Wn[p - 1] = rr.astype(np.int64), nn.astype(np.int64)

    # Per-core padded sizes (uniform across cores; one NEFF for all).
    Pp = np.zeros(NP + 1, np.int64)
    seg = [None] * (NP + 1)  # per pass: array of core segment starts (len NCORES+1)
    for p in range(1, NP + 1):
        s = np.searchsorted(Wr[p], np.arange(NCORES + 1) * RPC)
        seg[p] = s
        mx = int((s[1:] - s[:-1]).max()) if len(Wr[p]) else 0
        Pp[p] = max(8, -(-mx // 8) * 8)  # multiple of 8 so num_idxs % 16 == 0

    # Buffer slot bases: passes 1..NP-1 store 3 variants per update.
    base = np.zeros(NP + 1, np.int64)
    base[1] = NCONST
    for p in range(2, NP + 1):
        base[p] = base[p - 1] + 3 * Pp[p - 1]
    S = int(base[NP - 1] + 3 * Pp[NP - 1]) if NP >= 2 else NCONST
    assert S <= 32000, f"value buffer too large for int16 gather indices: {S}"

    # Per-(row,node) local slot index within its core's pass-p list.
    def local_ids(p):
        r = Wr[p]
        out = np.arange(len(r), dtype=np.int64)
        out -= seg[p][r // RPC]
        return out

    # Gather index arrays per pass, wrapped for ap_gather.
    idx_wrapped = []
    Ftot = 0
    slot_prev = np.full((B, N), -1, np.int64)
    for p in range(1, NP + 1):
        r, n = Wr[p], Wn[p]
        lj = local_ids(p)
        lch, rch = left[r, n], right[r, n]
        if p == 1:
            lidx = const_slot[r, lch]
            ridx = const_slot[r, rch]
        else:
            lidx = np.where(
                is_opa[r, lch],
                base[p - 1] + 3 * slot_prev[r, lch] + opsc[r, lch],
                const_slot[r, lch],
            )
            ridx = np.where(
                is_opa[r, rch],
                base[p - 1] + 3 * slot_prev[r, rch] + opsc[r, rch],
                const_slot[r, rch],
            )
        P = int(Pp[p])
        arr = np.full((NCORES, 2 * P), ZSLOT, np.int64)
        core = r // RPC
        arr[core, lj] = lidx
        arr[core, P + lj] = ridx
        F = -(-2 * P // 16)
        F += F & 1  # keep each pass idx slice 4-byte aligned (ucode reads dwords)
        tmp = np.full((NCORES, F * 16), ZSLOT, np.int64)
        tmp[:, : 2 * P] = arr
        idx_wrapped.append(tmp.reshape(NCORES, F, 16).transpose(0, 2, 1).astype(np.int16))
        Ftot += F
        if p < NP:
            slot_prev = np.full((B, N), -1, np.int64)
            slot_prev[r, n] = lj

    idx_full = np.concatenate(idx_wrapped, axis=2)  # (NCORES, 16, Ftot)

    return dict(
        Pp=Pp, base=base, S=S, idx=idx_full, Ftot=Ftot,
        r10=r10, seg10=seg[NP], lj10=local_ids(NP),
        opsc=opsc, litsc=litsc, is_lit=is_lit, m=m,
    )


def _build_nc(S, Pp, Ftot, P10):
    f32 = mybir.dt.float32
    nc = bacc.Bacc(None)
    consts = nc.dram_tensor("consts", [NI, NCONST], f32, kind="ExternalInput")
    wmat = nc.dram_tensor("wmat", [100, 74], f32, kind="ExternalInput")
    repl = nc.dram_tensor("repl", [NI, 100], f32, kind="ExternalInput")
    reprm = nc.dram_tensor("reprm", [NI, 100], f32, kind="ExternalInput")
    oblk = nc.dram_tensor("oblk", [74, NO], f32, kind="ExternalInput")
    oblk2 = nc.dram_tensor("oblk2", [NO, 74], f32, kind="ExternalInput")
    idx_in = nc.dram_tensor("idx", [16, Ftot], mybir.dt.int16, kind="ExternalInput")
    outz = nc.dram_tensor("outz", [74, P10], f32, kind="ExternalOutput")
    bufout = nc.dram_tensor("bufout", [16, S], f32, kind="ExternalOutput") if DEBUG_BUF else None

    with ExitStack() as ctx:
        tc = ctx.enter_context(tile.TileContext(nc))
        singles = ctx.enter_context(tc.tile_pool(name="singles", bufs=1))
        work = ctx.enter_context(tc.tile_pool(name="work", bufs=2))
        psum = ctx.enter_context(tc.tile_pool(name="psum", bufs=1, space="PSUM"))
        lrpool = ctx.enter_context(tc.tile_pool(name="lrpool", bufs=1))

        buf = singles.tile([16, S], f32)
        nc.vector.memset(buf[:, :], 0.0)
        nc.sync.dma_start(out=buf[0:NI, 0:NCONST], in_=consts[:, :])
        w_sb = singles.tile([100, 74], f32)
        nc.sync.dma_start(out=w_sb[:, :], in_=wmat[:, :])
        repl_sb = singles.tile([NI, 100], f32)
        nc.sync.dma_start(out=repl_sb[:, :], in_=repl[:, :])
        reprm_sb = singles.tile([NI, 100], f32)
        nc.sync.dma_start(out=reprm_sb[:, :], in_=reprm[:, :])
        oblk_sb = singles.tile([74, NO], f32)
        nc.sync.dma_start(out=oblk_sb[:, :], in_=oblk[:, :])
        oblk2_sb = singles.tile([NO, 74], f32)
        nc.sync.dma_start(out=oblk2_sb[:, :], in_=oblk2[:, :])
        idx_sb = singles.tile([16, Ftot], mybir.dt.int16)
        nc.sync.dma_start(out=idx_sb[:, :], in_=idx_in[:, :])

        foff = 0
        for p in range(1, NP + 1):
            P = int(Pp[p])
            F = -(-2 * P // 16)
            F += F & 1  # match _plan's 4-byte-aligned idx stride
            lr = lrpool.tile([16, 2 * P], f32, tag=f"lr{p}")
            nc.gpsimd.ap_gather(
                out_ap=lr[:, :],
                in_ap=buf[:, :],
                idxs_ap=idx_sb[:, foff : foff + F],
                channels=16,
                num_elems=S,
                d=1,
                num_idxs=2 * P,
            )
            foff += F
            for c0 in range(0, P, CHUNK):
                cw = min(CHUNK, P - c0)
                ps_l = psum.tile([100, cw], f32, tag="ps_l")
                nc.tensor.matmul(ps_l[:, :], repl_sb[:, :], lr[0:NI, c0 : c0 + cw],
                                 start=True, stop=True)
                ps_r = psum.tile([100, cw], f32, tag="ps_r")
                nc.tensor.matmul(ps_r[:, :], reprm_sb[:, :], lr[0:NI, P + c0 : P + c0 + cw],
                                 start=True, stop=True)
                lrep_sb = work.tile([100, cw], f32, tag="lrep_sb")
                nc.vector.tensor_copy(lrep_sb[:, :], ps_l[:, :])
                outer = work.tile([100, cw], f32, tag="outer")
                nc.vector.tensor_mul(outer[:, :], lrep_sb[:, :], ps_r[:, :])
                ps_z = psum.tile([74, cw], f32, tag="ps_z")
                nc.tensor.matmul(ps_z[:, :], w_sb[:, :], outer[:, :], start=True, stop=True)
                if p == NP:
                    zsb = work.tile([74, cw], f32, tag="zsb")
                    nc.scalar.copy(zsb[:, :], ps_z[:, :])
                    nc.sync.dma_start(out=outz[:, c0 : c0 + cw], in_=zsb[:, :])
                    continue
                e = work.tile([74, cw], f32, tag="e")
                nc.scalar.activation(e[:, :], ps_z[:, :], mybir.ActivationFunctionType.Exp)
                ps_z3 = psum.tile([NO, cw], f32, tag="ps_z3")
                nc.tensor.matmul(ps_z3[:, :], oblk_sb[:, :], e[:, :], start=True, stop=True)
                rz = work.tile([NO, cw], f32, tag="rz")
                nc.vector.reciprocal(rz[:, :], ps_z3[:, :])
                ps_rz = psum.tile([74, cw], f32, tag="ps_rz")
                nc.tensor.matmul(ps_rz[:, :], oblk2_sb[:, :], rz[:, :], start=True, stop=True)
                b0 = int(_CUR_BASE[p]) + 3 * c0
                for o in range(NO):
                    dst = buf[0:NI, b0 + o : b0 + 3 * cw : 3]
                    nc.vector.tensor_mul(
                        dst,
                        e[o * 32 : o * 32 + NI, :],
                        ps_rz[o * 32 : o * 32 + NI, :],
                    )
        if bufout is not None:
            nc.sync.dma_start(out=bufout[:, :], in_=buf[:, :])
    nc.finalize()
    return nc


_CUR_BASE = None  # set by kernel() before _build_nc (slot base per pass)


def kernel(op_table, cats, ops, lits, left, right, mask):
    global _CUR_BASE, LAST_RESULTS
    op_table = np.asarray(op_table, np.float32)
    plan = _plan(np.asarray(cats), np.asarray(ops), np.asarray(lits),
                 np.asarray(left), np.asarray(right), np.asarray(mask))
    Pp, base, S, Ftot = plan["Pp"], plan["base"], plan["S"], plan["Ftot"]
    P10 = int(Pp[NP])
    _CUR_BASE = base

    nc = _build_nc(S, Pp, Ftot, P10)

    consts = np.concatenate([np.eye(NI, dtype=np.float32),
                             np.zeros((NI, 1), np.float32)], axis=1)
    wmat = np.zeros((100, 74), np.float32)
    w30 = op_table.transpose(1, 2, 0, 3).reshape(100, 30)
    oblk = np.zeros((74, NO), np.float32)
    oblk2 = np.zeros((NO, 74), np.float32)
    for o in range(NO):
        wmat[:, o * 32 : o * 32 + NI] = w30[:, o * NI : (o + 1) * NI]
        oblk[o * 32 : o * 32 + NI, o] = 1.0
        oblk2[o, o * 32 : o * 32 + NI] = 1.0
    repl = np.kron(np.eye(NI), np.ones((1, NI))).astype(np.float32)
    reprm = np.tile(np.eye(NI), (1, NI)).astype(np.float32)

    in_maps = []
    for c in range(NCORES):
        in_maps.append({
            "consts": consts, "wmat": wmat, "repl": repl, "reprm": reprm,
            "oblk": oblk, "oblk2": oblk2,
            "idx": np.ascontiguousarray(plan["idx"][c]),
        })

    res = run_bass_kernel_spmd(nc, in_maps, list(range(NCORES)), trace=TRACE)
    LAST_RESULTS = res

    # Assemble the full (B, NI) output on the host (index selection only).
    out = np.zeros((B, NI), np.float32)
    litsc, is_lit, m = plan["litsc"], plan["is_lit"], plan["m"]
    lit_rows = np.nonzero(cats[:, 0] == 0)[0]
    lr_active = is_lit[lit_rows, 0]
    oh = 10.0 * np.eye(NI, dtype=np.float32)[litsc[lit_rows, 0]]
    out[lit_rows] = np.where(lr_active[:, None], oh, 0.0)

    r10, lj10, opsc = plan["r10"], plan["lj10"], plan["opsc"]
    for c in range(NCORES):
        z = np.asarray(res.results[c]["outz"])  # (30, P10)
        s0, s1 = plan["seg10"][c], plan["seg10"][c + 1]
        rows = r10[s0:s1]
        cols = lj10[s0:s1]
        o = opsc[rows, 0]
        zc = z[:, cols]  # (74, cnt)
        sel = np.stack([zc[i * 32 : i * 32 + NI, :] for i in range(NO)])  # (3,10,cnt)
        out[rows] = sel[o, :, np.arange(len(rows))]
    return out
